# revision 1
# baseline (speedup 1.0000x reference)
"""Segment-mean GNN aggregation (MeanAggregator) on 8 TRN2 NeuronCores.

out[v] = mean over edges (u -> v) of x[u], zeros for isolated nodes.

Strategy: shard destination nodes across the 8 cores (12500 each) and
replicate x (stored fp16) in every core's DRAM. The kernel is bound by
the SWDGE dma_gather descriptor rate (~2.1 ns per gathered row at 4
queues), so the design minimizes gathered slots:

- Each core's edges are bucketed into (segment, bank) cells, where a
  segment is 4 consecutive 128-dst groups and a bank is a 25000-row
  src window (dma_gather's int16 reach). Cell streams are sorted by
  (group, src) and padded to the 16-granular max across cores (~6%
  padding instead of the 25% that per-(group,bank) 128-aligned chunks
  would cost). Pads fetch row 0 (runtime index trimming measured slower
  than just fetching).
- Each bank's cells concatenate into one stream, gathered by ramped
  multi-packet ops (256/768/2048 then 4096; a cold op larger than the
  ring drains on its queue alone, stalling the Pool engine) with each
  bank bound to its own SWDGE queue; the final op is split so the tail
  drain spreads across queues. Per-bank head/tail index tiles keep the
  first gather from waiting on the bulk index-table DMA (tile-granular
  dependencies).
- A "chunk" is a static 128-slot window of a bank stream. Group
  boundaries inside cells vary per core, so a chunk may hold edges of
  2+ groups; the host computes the union over cores of (chunk, group)
  incidences, and the program runs one matmul per such pair. The
  per-core S matrix (one-hot slot ->  dst-in-group, -1 elsewhere)
  routes each edge to its dst and zeroes other groups' rows and pads.
- VectorE builds S columns (slot == iota) in 64-pair batches; TensorE
  accumulates S.T @ E into a PSUM tile per group; ScalarE copies PSUM
  scaled by 1/max(deg,1) into an 8-group tile; one DMA stores 8 groups.
"""

import math
from contextlib import ExitStack

import numpy as np

import concourse.tile as tile
from concourse import bacc, mybir
from concourse.bass_utils import run_bass_kernel_spmd

N_NODES = 100000
N_FEAT = 128
N_CORES = 8
NODES_PER_CORE = N_NODES // N_CORES  # 12500
P = 128
N_GROUPS = math.ceil(NODES_PER_CORE / P)  # 98
SEG = 4  # groups per segment cell
N_SEGS = math.ceil(N_GROUPS / SEG)  # 25
N_BANKS = 4
BANK = N_NODES // N_BANKS  # 25000
OP_IDX = 4096
SBATCH = 64  # pairs per IS_EQ batch
STORE_GROUPS = 8

_compiled_cache = {}


def _plan(cell_len, pair_key):
    """Static structure. cell_len: (N_SEGS, N_BANKS) 16-granular stream
    lengths (max over cores). pair_key: tuple of (g, b, chunk-in-bank)
    matmul pairs in emission order."""
    cell_len = np.asarray(cell_len)
    bank_len = [int(-128 * (-cell_len[:, b].sum() // 128))
                for b in range(N_BANKS)]
    cell_off = np.zeros((N_SEGS, N_BANKS), np.int64)
    for b in range(N_BANKS):
        off = 0
        for s in range(N_SEGS):
            cell_off[s, b] = off
            off += int(cell_len[s, b])
    bank_chunks = [L // 128 for L in bank_len]
    # op windows per bank stream: small ramp so all 4 SWDGE queues engage
    # before the big ops (a cold op drains on its queue alone at ~8.5
    # ns/desc while it exceeds ring space, stalling the Pool engine)
    bank_ops = []
    for b in range(N_BANKS):
        sizes = []
        done = 0
        ramp = [256, 768, 2048]
        while done < bank_len[b]:
            want = ramp.pop(0) if ramp else OP_IDX
            n = min(want, bank_len[b] - done)
            sizes.append(n)
            done += n
        # split the final op so the tail drain spreads across queues
        if len(sizes) > 4 and sizes[-1] > 1024:
            last = sizes.pop()
            h = (last // 2 // 128) * 128
            sizes.extend([last - h, h])
        ops = []
        done = 0
        for n in sizes:
            ops.append((done, n))
            done += n
        bank_ops.append(ops)
    pairs = [tuple(p) for p in pair_key]
    return {
        "cell_len": cell_len,
        "cell_off": cell_off,
        "bank_len": bank_len,
        "bank_chunks": bank_chunks,
        "bank_ops": bank_ops,
        "pairs": pairs,
    }


def _build_kernel(key):
    cell_len_key, pair_key = key
    plan = _plan(np.asarray(cell_len_key).reshape(N_SEGS, N_BANKS), pair_key)
    bank_len = plan["bank_len"]
    bank_ops = plan["bank_ops"]
    pairs = plan["pairs"]
    n_pairs = len(pairs)
    total_len = sum(bank_len)
    bank_stream_off = np.concatenate([[0], np.cumsum(bank_len)])

    nc = bacc.Bacc("TRN2", target_bir_lowering=False, debug=False,
                   num_devices=N_CORES, num_swdge_queues=4)
    f32, f16 = mybir.dt.float32, mybir.dt.float16
    x_d = nc.dram_tensor("x", [N_NODES, N_FEAT], f16,
                         kind="ExternalInput").ap()
    head_ds = [nc.dram_tensor(f"mhead{b}", [P, 16], mybir.dt.int16,
                              kind="ExternalInput").ap()
               for b in range(N_BANKS)]
    tail_ds = [nc.dram_tensor(f"mtail{b}", [P, (bank_len[b] - 256) // 16],
                              mybir.dt.int16, kind="ExternalInput").ap()
               for b in range(N_BANKS)]
    slot_d = nc.dram_tensor("mslot", [P, n_pairs], f16,
                            kind="ExternalInput").ap()
    invd_d = nc.dram_tensor("minvd", [P, N_GROUPS], f32,
                            kind="ExternalInput").ap()
    iota_d = nc.dram_tensor("miota", [P, SBATCH * P], f16,
                            kind="ExternalInput").ap()
    out_d = nc.dram_tensor("out", [NODES_PER_CORE, N_FEAT], f32,
                           kind="ExternalOutput").ap()

    with tile.TileContext(nc) as tc, ExitStack() as ctx:
        meta_pool = ctx.enter_context(tc.tile_pool(name="meta", bufs=1))
        head_ts = []
        for b in range(N_BANKS):
            t = meta_pool.tile([P, 16], mybir.dt.int16, tag=f"head{b}")
            nc.sync.dma_start(out=t[:], in_=head_ds[b][:])
            head_ts.append(t)
        slot_t = meta_pool.tile([P, n_pairs], f16)
        nc.sync.dma_start(out=slot_t[:], in_=slot_d[:])
        iota_t = meta_pool.tile([P, SBATCH * P], f16)
        nc.sync.dma_start(out=iota_t[:], in_=iota_d[:])
        tail_ts = []
        for b in range(N_BANKS):
            t = meta_pool.tile([P, (bank_len[b] - 256) // 16],
                               mybir.dt.int16, tag=f"tail{b}")
            nc.sync.dma_start(out=t[:], in_=tail_ds[b][:])
            tail_ts.append(t)
        invd_t = meta_pool.tile([P, N_GROUPS], f32)
        nc.sync.dma_start(out=invd_t[:], in_=invd_d[:])

        gat_pool = ctx.enter_context(tc.tile_pool(name="gat", bufs=12))
        sel_pool = ctx.enter_context(tc.tile_pool(name="sel", bufs=3))
        psum_pool = ctx.enter_context(
            tc.tile_pool(name="psum", bufs=8, space="PSUM"))
        out_pool = ctx.enter_context(tc.tile_pool(name="outb", bufs=3))

        op_counter = [0]
        emitted_ops = [0] * N_BANKS
        op_tiles = {}  # (b, op_i) -> tile
        # chunk -> (op index, column within op tile) per bank
        chunk_op = []
        for b in range(N_BANKS):
            m = {}
            for oi, (off, n) in enumerate(bank_ops[b]):
                for j in range(n // 128):
                    m[off // 128 + j] = (oi, j)
            chunk_op.append(m)

        def emit_ops_until(b, chunk):
            """Emit bank-b gather ops until the op holding `chunk` exists."""
            need_op = chunk_op[b][chunk][0]
            while emitted_ops[b] <= need_op:
                oi = emitted_ops[b]
                off, n = bank_ops[b][oi]
                g_t = gat_pool.tile([P, OP_IDX // 128, N_FEAT], f16,
                                    tag="gat")
                if off == 0:
                    src_idx = head_ts[b][:, :16]
                else:
                    src_idx = tail_ts[b][:, (off - 256) // 16:
                                         (off - 256 + n) // 16]
                nc.gpsimd.dma_gather(
                    out_ap=g_t[:, :n // 128, :],
                    in_ap=x_d[b * BANK:(b + 1) * BANK, :],
                    idxs_ap=src_idx,
                    num_idxs=n,
                    num_idxs_reg=n,
                    elem_size=N_FEAT,
                    queue_num=b,
                    single_packet=False,
                )
                op_tiles[(b, oi)] = g_t
                op_counter[0] += 1
                emitted_ops[b] += 1

        s_tiles = {}

        def emit_sbatch(bi):
            c0 = bi * SBATCH
            n = min(SBATCH, n_pairs - c0)
            s_t = sel_pool.tile([P, SBATCH * P], f16, tag="sel")
            nc.vector.tensor_tensor(
                out=s_t[:, :n * P],
                in0=slot_t[:, c0:c0 + n].unsqueeze(2).to_broadcast([P, n, P]),
                in1=iota_t[:, :n * P].rearrange("p (a b) -> p a b", a=n),
                op=mybir.AluOpType.is_equal,
            )
            s_tiles[bi] = s_t

        # group -> list of pair indices (in emission order)
        group_pairs = {}
        for pi, (g, b, c) in enumerate(pairs):
            group_pairs.setdefault(g, []).append(pi)

        # prefetch: emit first ops of every bank to prime the queues
        for b in range(N_BANKS):
            emit_ops_until(b, 0)

        out_t = None
        for g in range(N_GROUPS):
            plist = group_pairs[g]
            # prefetch gathers a segment ahead
            if g + SEG < N_GROUPS:
                for pi in group_pairs[g + SEG]:
                    _, b, c = pairs[pi]
                    emit_ops_until(b, c)
            ps = psum_pool.tile([P, N_FEAT], f32)
            for i, pi in enumerate(plist):
                _, b, c = pairs[pi]
                emit_ops_until(b, c)
                bi = pi // SBATCH
                if bi not in s_tiles:
                    emit_sbatch(bi)
                if (bi + 1) * SBATCH < n_pairs and bi + 1 not in s_tiles:
                    emit_sbatch(bi + 1)
                s_t = s_tiles[bi]
                lc = pi - bi * SBATCH
                oi, col = chunk_op[b][c]
                g_t = op_tiles[(b, oi)]
                nc.tensor.matmul(
                    ps[:],
                    lhsT=s_t[:, lc * P:(lc + 1) * P],
                    rhs=g_t[:, col, :],
                    start=(i == 0),
                    stop=(i == len(plist) - 1),
                )
            if g % STORE_GROUPS == 0:
                out_t = out_pool.tile([P, STORE_GROUPS, N_FEAT], f32,
                                      tag="outb")
            nc.scalar.activation(out=out_t[:, g % STORE_GROUPS, :], in_=ps[:],
                                 func=mybir.ActivationFunctionType.Copy,
                                 scale=invd_t[:, g:g + 1])
            if g % STORE_GROUPS == STORE_GROUPS - 1 or g == N_GROUPS - 1:
                g0 = (g // STORE_GROUPS) * STORE_GROUPS
                ngroups = g - g0 + 1
                nfull = ngroups
                rows_last = min(P, NODES_PER_CORE - (g0 + ngroups - 1) * P)
                if rows_last < P:
                    nfull -= 1
                if nfull > 0:
                    dst = out_d[g0 * P:(g0 + nfull) * P, :].rearrange(
                        "(j p) f -> p j f", p=P)
                    nc.sync.dma_start(out=dst, in_=out_t[:, :nfull, :])
                if nfull < ngroups:
                    gl = g0 + ngroups - 1
                    nc.sync.dma_start(
                        out=out_d[gl * P:gl * P + rows_last, :],
                        in_=out_t[:rows_last, ngroups - 1, :])
    nc.compile()
    return nc


def _prepare(x, edge_src, edge_dst):
    x16 = np.ascontiguousarray(np.asarray(x), dtype=np.float16)
    src = np.asarray(edge_src).astype(np.int64)
    dst = np.asarray(edge_dst).astype(np.int64)

    deg = np.bincount(dst, minlength=N_NODES)
    inv_deg = (1.0 / np.maximum(deg, 1)).astype(np.float32)

    core_e = dst // NODES_PER_CORE
    ldst = dst % NODES_PER_CORE
    g_e = ldst // P
    s_e = g_e // SEG
    b_e = src // BANK

    cnt = np.zeros((N_CORES, N_SEGS, N_BANKS), np.int64)
    np.add.at(cnt, (core_e, s_e, b_e), 1)
    cell_len = np.asarray(-16 * (-(cnt.max(axis=0)) // 16), np.int64)

    plan = _plan(cell_len, ())
    cell_off = plan["cell_off"]
    bank_len = plan["bank_len"]
    bank_stream_off = np.concatenate([[0], np.cumsum(bank_len)])
    total_len = int(bank_stream_off[-1])

    # per-core packing: stream position of every edge + group runs
    per_core = []
    pair_set = [set() for _ in range(N_GROUPS)]
    for k in range(N_CORES):
        m = core_e == k
        ksrc, kg, kb, ks = src[m], g_e[m], b_e[m], s_e[m]
        kldst = ldst[m]
        order = np.lexsort((ksrc, kg, kb, ks))
        ksrc, kg, kb, ks, kldst = (ksrc[order], kg[order], kb[order],
                                   ks[order], kldst[order])
        # stream position: cell base + rank within cell
        cid = (ks * N_BANKS + kb)
        # rank within cell: edges are sorted by (s, b, g, src) so within a
        # cell they are consecutive
        pos = np.zeros(len(ksrc), np.int64)
        uniq, starts, counts = np.unique(cid, return_index=True,
                                         return_counts=True)
        for u, st, n in zip(uniq, starts, counts):
            s, b = int(u) // N_BANKS, int(u) % N_BANKS
            assert n <= cell_len[s, b]
            pos[st:st + n] = cell_off[s, b] + np.arange(n)
        chunk = pos // 128  # chunk index within the bank stream
        for g in range(N_GROUPS):
            gm = kg == g
            if not gm.any():
                continue
            b_of = kb[gm]
            ch = chunk[gm]
            for b, c in set(zip(b_of.tolist(), ch.tolist())):
                pair_set[g].add((b, c))
        per_core.append((ksrc, kg, kb, kldst, pos))

    # pair list in emission order (group-major, then bank, then chunk)
    pairs = []
    pair_index = {}
    for g in range(N_GROUPS):
        cells = sorted(pair_set[g])
        if not cells:
            cells = [(0, 0)]  # dummy pair so psum/out get written
        for b, c in cells:
            pair_index[(g, b, c)] = len(pairs)
            pairs.append((g, b, c))
    pair_key = tuple(pairs)
    n_pairs = len(pairs)

    iota = np.tile(np.arange(P, dtype=np.float16)[None, :], (P, SBATCH))
    in_maps = []
    for k in range(N_CORES):
        ksrc, kg, kb, kldst, pos = per_core[k]
        idx_flat = np.zeros(total_len, np.int16)
        gpos = bank_stream_off[kb] + pos
        idx_flat[gpos] = (ksrc - kb * BANK).astype(np.int16)
        heads, tails = {}, {}
        for b in range(N_BANKS):
            st = idx_flat[bank_stream_off[b]:bank_stream_off[b + 1]]
            heads[f"mhead{b}"] = np.ascontiguousarray(
                np.tile(st[:256].reshape(-1, 16).T, (8, 1)))
            tails[f"mtail{b}"] = np.ascontiguousarray(
                np.tile(st[256:].reshape(-1, 16).T, (8, 1)))
        slot_tab = np.full((n_pairs, P), -1.0, np.float16)
        pidx = np.fromiter(
            (pair_index[(g, b, c)] for g, b, c in
             zip(kg.tolist(), kb.tolist(), (pos // 128).tolist())),
            np.int64, len(kg))
        slot_tab[pidx, pos % 128] = (kldst - kg * P).astype(np.float16)
        invd = np.zeros((N_GROUPS * P,), np.float32)
        invd[:NODES_PER_CORE] = inv_deg[k * NODES_PER_CORE:
                                        (k + 1) * NODES_PER_CORE]
        in_maps.append({
            "x": x16,
            **heads,
            **tails,
            "mslot": np.ascontiguousarray(slot_tab.T),
            "minvd": np.ascontiguousarray(invd.reshape(N_GROUPS, P).T),
            "miota": np.ascontiguousarray(iota),
        })
    key = (tuple(int(v) for v in cell_len.ravel()), pair_key)
    kernel.last_stats = {"total_len": total_len, "n_pairs": n_pairs}
    return in_maps, key


def kernel(x, edge_src, edge_dst, _trace=False):
    in_maps, key = _prepare(x, edge_src, edge_dst)
    nc = _compiled_cache.get(key)
    if nc is None:
        nc = _build_kernel(key)
        _compiled_cache[key] = nc
    res = run_bass_kernel_spmd(nc, in_maps, core_ids=list(range(N_CORES)),
                               trace=_trace)
    out = np.concatenate([res.results[k]["out"] for k in range(N_CORES)],
                         axis=0)
    if _trace:
        kernel.last_exec_time_ns = res.exec_time_ns
        kernel.last_result = res
    return out

